# revision 21
# baseline (speedup 1.0000x reference)
"""EuclideanCodebook (VQ codebook lookup) Trainium2 Bass kernel.

Problem (hardcoded shapes):
  x:             [8, 4096, 256] f32
  embedding_sum: [2048, 256]    f32
  cluster_usage: [2048]         f32
Returns (quantized [8,4096,256] f32, codes [8,4096] int32) — matching
  emb   = embedding_sum / clamp(cluster_usage, 1e-5)
  codes = argmin_k ||x - emb_k||^2
  quantized = emb[codes]

Strategy: pure data-parallel over the flattened N=32768 points; each of the
8 cores handles 4096 points.  argmin_k d = argmax_k (x.e_k - 0.5|e_k|^2); the
|x|^2 term is constant per point and dropped.

Matmul precision/speed: bf16 matmuls stream at 1 cycle/row on the PE (fp32 is
4, f32r is 2 — measured).  Full fp32-level argmin accuracy is recovered with
a 3-term split  x.e ~= xh.eh + xl.eh + xh.el  where xh/eh are bf16 roundings
and xl/el bf16 roundings of the residuals (missing terms are O(2^-24); this
reproduces the fp32 reference argmin exactly on the benchmark data).
The -0.5|e|^2 bias rides the same PSUM accumulation as a contraction-3
matmul of ones against a 3-way bf16 split of the bias.

Per 128-point tile:
  - ACT rounds x to xh (bf16), DVE computes xl = x - xh (bf16)
  - PE transposes xh/xl to [D, pts] (bf16 via identity matmul)
  - PE: 7 bf16 matmuls per 512-wide PSUM bank (scores), 4 banks
  - ACT evacuates PSUM -> SBUF
  - DVE max + max_index give the argmax code per point
  - GPSIMD indirect-DMA gathers emb[codes] from a DRAM scratch copy of emb
"""

import subprocess
import sys

import numpy as np

import concourse.bass as bass
import concourse.mybir as mybir
import concourse.tile as tile
from concourse import bacc
from concourse.bass_utils import run_bass_kernel_spmd
from concourse.masks import make_identity

B, T, D, K = 8, 4096, 256, 2048
N_CORES = 8
P = 128
EPS = 1e-5

F32 = mybir.dt.float32
BF16 = mybir.dt.bfloat16
U32 = mybir.dt.uint32
I32 = mybir.dt.int32

KO = K // P  # 16 k-chunks
DC = D // P  # 2 contraction chunks
NB = K // 512  # 4 psum banks of 512 scores

ACT_COPY = mybir.ActivationFunctionType.Copy
ACT_SQUARE = mybir.ActivationFunctionType.Square


def _body(tc, x, es, cu, q, codes, emb_dram, b3_dram, ntiles):
    nc = tc.nc

    with (
        tc.tile_pool(name="const", bufs=1) as const,
        tc.tile_pool(name="xin", bufs=4) as xpool,
        tc.tile_pool(name="xnat", bufs=3) as xnpool,
        tc.tile_pool(name="xT", bufs=3) as xTpool,
        tc.tile_pool(name="scores", bufs=2) as scpool,
        tc.tile_pool(name="qout", bufs=4) as qpool,
        tc.tile_pool(name="small", bufs=6) as smallpool,
        tc.tile_pool(name="psum_t", bufs=2, space="PSUM") as psum_t,
    ):
        # ---------------- preamble: emb, emb^T (bf16 hi/lo), bias ----------
        # k-chunk layout: [p, o] maps to code k = o*128 + p, so score column
        # l (= o*128 + p within the 2048-wide score row) IS the code index.
        usage_op = const.tile([KO, P], F32)
        nc.sync.dma_start(usage_op, cu.rearrange("(o p) -> o p", p=P))
        inv = const.tile([P, KO], F32)

        ones3 = const.tile([3, P], BF16)
        nc.vector.memset(ones3, 1.0)

        ident = const.tile([P, P], BF16)
        make_identity(nc, ident)
        ident_f32 = const.tile([P, P], F32)
        make_identity(nc, ident_f32)

        ptu = psum_t.tile([P, KO], F32, tag="pt1")
        nc.tensor.transpose(ptu, usage_op, ident_f32[:KO, :KO])
        usage = const.tile([P, KO], F32)
        nc.scalar.copy(usage, ptu)
        nc.vector.tensor_scalar_max(inv, usage, EPS)
        nc.vector.reciprocal(inv, inv)


        es_kp = const.tile([P, KO, D], F32)
        es_view = es.rearrange("(o p) d -> o p d", p=P)
        for o in range(KO):
            nc.sync.dma_start(es_kp[:, o, :], es_view[o])
        emb_kp = const.tile([P, KO, D], F32)
        for o in range(KO):
            # emb = embedding_sum * (1/clamped_usage), per-partition scale
            nc.vector.tensor_scalar_mul(
                emb_kp[:, o, :], es_kp[:, o, :], inv[:, o : o + 1]
            )
        # gather table in true-k order
        emb_view = emb_dram.rearrange("(o p) d -> o p d", p=P)
        for o in range(KO):
            nc.sync.dma_start(emb_view[o], emb_kp[:, o, :])

        # esq[p, o] = (sum_d es^2) * inv^2 ; bias = -0.5 * esq laid out [1, K].
        # Squaring unscaled es decouples the bias chain from the emb scaling.
        sq_scr = const.tile([P, D], F32)
        esq_po = const.tile([P, KO], F32)
        for o in range(KO):
            nc.scalar.activation(
                sq_scr, es_kp[:, o, :], ACT_SQUARE, accum_out=esq_po[:, o : o + 1]
            )
        inv2 = const.tile([P, KO], F32)
        nc.vector.tensor_mul(inv2, inv, inv)
        nc.vector.tensor_mul(esq_po, esq_po, inv2)
        nc.vector.tensor_scalar_mul(esq_po, esq_po, -0.5)
        # 3-way bf16 split of the bias, done in [128, KO] layout (fast), then
        # assembled into [3, K] k-order rows via strided DMAs.
        bh_po = const.tile([P, KO], BF16)
        nc.scalar.copy(bh_po, esq_po)
        r1_po = const.tile([P, KO], F32)
        nc.vector.tensor_sub(r1_po, esq_po, bh_po)
        bm_po = const.tile([P, KO], BF16)
        nc.scalar.copy(bm_po, r1_po)
        bl_po = const.tile([P, KO], BF16)
        nc.vector.tensor_sub(bl_po, r1_po, bm_po)
        # transpose each [128, KO] split to [KO, 128] so the DRAM write is
        # contiguous per o-chunk (k = o*128 + p order)
        for r, row_po in enumerate((bh_po, bm_po, bl_po)):
            ptb = psum_t.tile([KO, P], BF16, tag="pt1")
            nc.tensor.transpose(ptb, row_po, ident)
            stg = const.tile([KO, P], BF16, name=f"bstg_{r}")
            nc.scalar.copy(stg, ptb)
            nc.sync.dma_start(b3_dram[r, :].rearrange("(o p) -> o p", p=P), stg)
        bias3 = const.tile([3, K], BF16)
        nc.sync.dma_start(bias3, b3_dram)

        # emb^T [d, dc, l] as bf16 hi (eh) and residual (el): transpose emb in
        # fp32 on the PE, then eh = round(embT) on ACT, el = embT - eh on DVE.
        eh_b = [const.tile([P, DC, 512], BF16, name=f"eh_{b}") for b in range(NB)]
        el_b = [const.tile([P, DC, 512], BF16, name=f"el_{b}") for b in range(NB)]
        pre_pt_pool = tc.tile_pool(name="pre_pt", bufs=4, space="PSUM")
        pre_pt = pre_pt_pool.__enter__()
        for o in range(KO):
            b, oo = o // 4, o % 4
            for dc in range(DC):
                pt = pre_pt.tile([P, P], F32, tag="pt")
                nc.tensor.transpose(pt, emb_kp[:, o, dc * P : (dc + 1) * P], ident_f32)
                ehs = eh_b[b][:, dc, oo * P : (oo + 1) * P]
                nc.scalar.copy(ehs, pt)
                nc.vector.tensor_sub(el_b[b][:, dc, oo * P : (oo + 1) * P], pt, ehs)
        pre_pt_pool.__exit__(None, None, None)
        psum_s_pool = tc.tile_pool(name="psum_s", bufs=4, space="PSUM")
        psum_s = psum_s_pool.__enter__()

        # ---------------- main loop over 128-point tiles -------------------
        for t in range(ntiles):
            xt = xpool.tile([P, D], F32)
            nc.sync.dma_start(xt, x[t * P : (t + 1) * P, :])

            xh_nat = xnpool.tile([P, D], BF16, tag="xh_nat")
            nc.scalar.copy(xh_nat, xt)
            xl_nat = xnpool.tile([P, D], BF16, tag="xl_nat")
            nc.gpsimd.tensor_sub(xl_nat, xt, xh_nat)

            xh = xTpool.tile([P, DC, P], BF16, tag="xh")
            xl = xTpool.tile([P, DC, P], BF16, tag="xl")
            for dc in range(DC):
                pt1 = psum_t.tile([P, P], BF16, tag="pt1")
                nc.tensor.transpose(pt1, xh_nat[:, dc * P : (dc + 1) * P], ident)
                nc.scalar.copy(xh[:, dc, :], pt1)
                pt2 = psum_t.tile([P, P], BF16, tag="pt2")
                nc.tensor.transpose(pt2, xl_nat[:, dc * P : (dc + 1) * P], ident)
                nc.scalar.copy(xl[:, dc, :], pt2)

            sc = scpool.tile([P, K], F32)
            for nb in range(NB):
                terms = [(xh, eh_b[nb]), (xl, eh_b[nb]), (xh, el_b[nb])]
                bank = psum_s.tile([P, 512], F32, tag="bank")
                # bias first: its extra cost merges with the start=True penalty
                nc.tensor.matmul(
                    bank,
                    ones3,
                    bias3[:, nb * 512 : (nb + 1) * 512],
                    start=True,
                    stop=False,
                )
                n_mm = len(terms) * DC
                i = 0
                for lhsT, rhs in terms:
                    for dc in range(DC):
                        i += 1
                        nc.tensor.matmul(
                            bank,
                            lhsT[:, dc, :],
                            rhs[:, dc, :],
                            start=False,
                            stop=(i == n_mm),
                        )
                nc.scalar.copy(sc[:, nb * 512 : (nb + 1) * 512], bank)

            mx = smallpool.tile([P, 8], F32)
            nc.vector.max(out=mx, in_=sc)
            idx = smallpool.tile([P, 8], U32)
            nc.vector.max_index(out=idx, in_max=mx, in_values=sc)

            nc.sync.dma_start(
                codes[t * P : (t + 1) * P, None], idx[:, 0:1].bitcast(I32)
            )

            qt = qpool.tile([P, D], F32)
            nc.gpsimd.indirect_dma_start(
                out=qt,
                out_offset=None,
                in_=emb_dram[:, :],
                in_offset=bass.IndirectOffsetOnAxis(ap=idx[:, 0:1], axis=0),
            )
            nc.sync.dma_start(q[t * P : (t + 1) * P, :], qt)

        psum_s_pool.__exit__(None, None, None)


def build_bass(nshard):
    """Build + bacc-compile the single-core SPMD program for an nshard-point
    shard. Returns the Bass object."""
    assert nshard % P == 0
    nc = bacc.Bacc(
        "TRN2",
        target_bir_lowering=False,
        debug=False,
        enable_asserts=False,
        num_devices=N_CORES,
    )
    x = nc.dram_tensor("x", [nshard, D], F32, kind="ExternalInput")
    es = nc.dram_tensor("embedding_sum", [K, D], F32, kind="ExternalInput")
    cu = nc.dram_tensor("cluster_usage", [K], F32, kind="ExternalInput")
    q = nc.dram_tensor("quantized", [nshard, D], F32, kind="ExternalOutput")
    codes = nc.dram_tensor("codes", [nshard], I32, kind="ExternalOutput")
    emb_dram = nc.dram_tensor("emb_scratch", [K, D], F32)
    b3_dram = nc.dram_tensor("b3_scratch", [3, K], BF16)

    with tile.TileContext(nc) as tc:
        _body(
            tc,
            x.ap(),
            es.ap(),
            cu.ap(),
            q.ap(),
            codes.ap(),
            emb_dram.ap(),
            b3_dram.ap(),
            nshard // P,
        )
    nc.compile()
    return nc


_CACHE = {}
_DEVICE_POKED = False


def _poke_device():
    """A bass NEFF execution can leave the axon terminal worker in an
    unrecoverable state for the NEXT session; the first device touch from a
    fresh process fails once and restarts the worker. Absorb that in a
    sacrificial subprocess before doing real work in this process."""
    global _DEVICE_POKED
    if _DEVICE_POKED:
        return
    for _ in range(3):
        try:
            r = subprocess.run(
                [
                    sys.executable,
                    "-c",
                    "import jax, jax.numpy as jnp; "
                    "(jnp.ones((4,4)) @ jnp.ones((4,4))).block_until_ready()",
                ],
                capture_output=True,
                timeout=180,
            )
            if r.returncode == 0:
                break
        except subprocess.TimeoutExpired:
            pass
    _DEVICE_POKED = True


def _get_nc(nshard):
    if nshard not in _CACHE:
        _CACHE[nshard] = build_bass(nshard)
    return _CACHE[nshard]


def run_sharded(x, embedding_sum, cluster_usage, trace=False, nc=None):
    """Run on 8 NeuronCores; returns (quantized, codes, BassKernelResults)."""
    x = np.ascontiguousarray(np.asarray(x, dtype=np.float32))
    es = np.ascontiguousarray(np.asarray(embedding_sum, dtype=np.float32))
    cu = np.ascontiguousarray(np.asarray(cluster_usage, dtype=np.float32))
    flat = x.reshape(-1, x.shape[-1])
    n = flat.shape[0]
    assert n % N_CORES == 0
    nshard = n // N_CORES
    _poke_device()
    if nc is None:
        nc = _get_nc(nshard)

    in_maps = [
        {
            "x": np.ascontiguousarray(flat[c * nshard : (c + 1) * nshard]),
            "embedding_sum": es,
            "cluster_usage": cu,
        }
        for c in range(N_CORES)
    ]
    res = run_bass_kernel_spmd(nc, in_maps, core_ids=list(range(N_CORES)), trace=trace)
    quant = np.concatenate([r["quantized"] for r in res.results], axis=0)
    cds = np.concatenate([r["codes"] for r in res.results], axis=0)
    quant = quant.reshape(x.shape)
    cds = cds.reshape(x.shape[:-1])
    return quant, cds, res


def kernel(x, embedding_sum, cluster_usage):
    quant, cds, _ = run_sharded(x, embedding_sum, cluster_usage)
    return quant, cds


# revision 22
# speedup vs baseline: 1.1182x; 1.1182x over previous
"""EuclideanCodebook (VQ codebook lookup) Trainium2 Bass kernel.

Problem (hardcoded shapes):
  x:             [8, 4096, 256] f32
  embedding_sum: [2048, 256]    f32
  cluster_usage: [2048]         f32
Returns (quantized [8,4096,256] f32, codes [8,4096] int32) — matching
  emb   = embedding_sum / clamp(cluster_usage, 1e-5)
  codes = argmin_k ||x - emb_k||^2
  quantized = emb[codes]

Strategy: pure data-parallel over the flattened N=32768 points; each of the
8 cores handles 4096 points.  argmin_k d = argmax_k (x.e_k - 0.5|e_k|^2); the
|x|^2 term is constant per point and dropped.

Matmul precision/speed: bf16 matmuls stream at 1 cycle/row on the PE (fp32 is
4, f32r is 2 — measured).  Full fp32-level argmin accuracy is recovered with
a 3-term split  x.e ~= xh.eh + xl.eh + xh.el  where xh/eh are bf16 roundings
and xl/el bf16 roundings of the residuals (missing terms are O(2^-24); this
reproduces the fp32 reference argmin exactly on the benchmark data).
The -0.5|e|^2 bias rides the same PSUM accumulation as a contraction-3
matmul of ones against a 3-way bf16 split of the bias.

Per 128-point tile:
  - ACT rounds x to xh (bf16), DVE computes xl = x - xh (bf16)
  - PE transposes xh/xl to [D, pts] (bf16 via identity matmul)
  - PE: 7 bf16 matmuls per 512-wide PSUM bank (scores), 4 banks
  - ACT evacuates PSUM -> SBUF
  - DVE max + max_index give the argmax code per point
  - GPSIMD indirect-DMA gathers emb[codes] from a DRAM scratch copy of emb
"""

import subprocess
import sys

import numpy as np

import concourse.bass as bass
import concourse.mybir as mybir
import concourse.tile as tile
from concourse import bacc
from concourse.bass_utils import run_bass_kernel_spmd
from concourse.masks import make_identity

B, T, D, K = 8, 4096, 256, 2048
N_CORES = 8
P = 128
EPS = 1e-5

F32 = mybir.dt.float32
BF16 = mybir.dt.bfloat16
U32 = mybir.dt.uint32
I32 = mybir.dt.int32

KO = K // P  # 16 k-chunks
DC = D // P  # 2 contraction chunks
NB = K // 512  # 4 psum banks of 512 scores

ACT_COPY = mybir.ActivationFunctionType.Copy
ACT_SQUARE = mybir.ActivationFunctionType.Square


def _body(tc, x, es, cu, q, codes, emb_dram, b3_dram, ntiles):
    nc = tc.nc

    with (
        tc.tile_pool(name="const", bufs=1) as const,
        tc.tile_pool(name="xin", bufs=4) as xpool,
        tc.tile_pool(name="xnat", bufs=3) as xnpool,
        tc.tile_pool(name="xT", bufs=3) as xTpool,
        tc.tile_pool(name="scores", bufs=2) as scpool,
        tc.tile_pool(name="qout", bufs=4) as qpool,
        tc.tile_pool(name="small", bufs=6) as smallpool,
        tc.tile_pool(name="psum_t", bufs=2, space="PSUM") as psum_t,
    ):
        # ---------------- preamble: emb, emb^T (bf16 hi/lo), bias ----------
        # k-chunk layout: [p, o] maps to code k = o*128 + p, so score column
        # l (= o*128 + p within the 2048-wide score row) IS the code index.
        usage_op = const.tile([KO, P], F32)
        nc.sync.dma_start(usage_op, cu.rearrange("(o p) -> o p", p=P))
        inv = const.tile([P, KO], F32)

        ones3 = const.tile([3, P], BF16)
        nc.vector.memset(ones3, 1.0)

        ident = const.tile([P, P], BF16)
        make_identity(nc, ident)
        ident_f32 = const.tile([P, P], F32)
        make_identity(nc, ident_f32)

        ptu = psum_t.tile([P, KO], F32, tag="pt1")
        nc.tensor.transpose(ptu, usage_op, ident_f32[:KO, :KO])
        usage = const.tile([P, KO], F32)
        nc.scalar.copy(usage, ptu)
        nc.vector.tensor_scalar_max(inv, usage, EPS)
        nc.vector.reciprocal(inv, inv)


        es_kp = const.tile([P, KO, D], F32)
        es_view = es.rearrange("(o p) d -> o p d", p=P)
        for o in range(KO):
            nc.sync.dma_start(es_kp[:, o, :], es_view[o])
        emb_kp = const.tile([P, KO, D], F32)
        for o in range(KO):
            # emb = embedding_sum * (1/clamped_usage), per-partition scale
            nc.vector.tensor_scalar_mul(
                emb_kp[:, o, :], es_kp[:, o, :], inv[:, o : o + 1]
            )
        # gather table in true-k order
        emb_view = emb_dram.rearrange("(o p) d -> o p d", p=P)
        for o in range(KO):
            nc.sync.dma_start(emb_view[o], emb_kp[:, o, :])

        # esq[p, o] = (sum_d es^2) * inv^2 ; bias = -0.5 * esq laid out [1, K].
        # Squaring unscaled es decouples the bias chain from the emb scaling.
        sq_scr = const.tile([P, D], F32)
        esq_po = const.tile([P, KO], F32)
        for o in range(KO):
            nc.scalar.activation(
                sq_scr, es_kp[:, o, :], ACT_SQUARE, accum_out=esq_po[:, o : o + 1]
            )
        inv2 = const.tile([P, KO], F32)
        nc.vector.tensor_mul(inv2, inv, inv)
        nc.vector.tensor_mul(esq_po, esq_po, inv2)
        nc.vector.tensor_scalar_mul(esq_po, esq_po, -0.5)
        # 3-way bf16 split of the bias, done in [128, KO] layout (fast), then
        # assembled into [3, K] k-order rows via strided DMAs.
        bh_po = const.tile([P, KO], BF16)
        nc.scalar.copy(bh_po, esq_po)
        r1_po = const.tile([P, KO], F32)
        nc.vector.tensor_sub(r1_po, esq_po, bh_po)
        bm_po = const.tile([P, KO], BF16)
        nc.scalar.copy(bm_po, r1_po)
        bl_po = const.tile([P, KO], BF16)
        nc.vector.tensor_sub(bl_po, r1_po, bm_po)
        # transpose each [128, KO] split to [KO, 128] so the DRAM write is
        # contiguous per o-chunk (k = o*128 + p order)
        for r, row_po in enumerate((bh_po, bm_po, bl_po)):
            ptb = psum_t.tile([KO, P], BF16, tag="pt1")
            nc.tensor.transpose(ptb, row_po, ident)
            stg = const.tile([KO, P], BF16, name=f"bstg_{r}")
            nc.scalar.copy(stg, ptb)
            nc.sync.dma_start(b3_dram[r, :].rearrange("(o p) -> o p", p=P), stg)
        bias3 = const.tile([3, K], BF16)
        nc.sync.dma_start(bias3, b3_dram)

        # emb^T [d, dc, l] as bf16 hi (eh) and residual (el): transpose emb in
        # fp32 on the PE, then eh = round(embT) on ACT, el = embT - eh on DVE.
        eh_b = [const.tile([P, DC, 512], BF16, name=f"eh_{b}") for b in range(NB)]
        el_b = [const.tile([P, DC, 512], BF16, name=f"el_{b}") for b in range(NB)]
        pre_pt_pool = tc.tile_pool(name="pre_pt", bufs=4, space="PSUM")
        pre_pt = pre_pt_pool.__enter__()
        for o in range(KO):
            b, oo = o // 4, o % 4
            for dc in range(DC):
                pt = pre_pt.tile([P, P], F32, tag="pt")
                nc.tensor.transpose(pt, emb_kp[:, o, dc * P : (dc + 1) * P], ident_f32)
                ehs = eh_b[b][:, dc, oo * P : (oo + 1) * P]
                nc.scalar.copy(ehs, pt)
                nc.vector.tensor_sub(el_b[b][:, dc, oo * P : (oo + 1) * P], pt, ehs)
        pre_pt_pool.__exit__(None, None, None)
        psum_s_pool = tc.tile_pool(name="psum_s", bufs=4, space="PSUM")
        psum_s = psum_s_pool.__enter__()

        # ---------------- main loop over 128-point tiles -------------------
        for t in range(ntiles):
            xt = xpool.tile([P, D], F32)
            nc.sync.dma_start(xt, x[t * P : (t + 1) * P, :])

            xh_nat = xnpool.tile([P, D], BF16, tag="xh_nat")
            nc.scalar.copy(xh_nat, xt)
            xl_nat = xnpool.tile([P, D], BF16, tag="xl_nat")
            nc.gpsimd.tensor_sub(xl_nat, xt, xh_nat)

            xh = xTpool.tile([P, DC, P], BF16, tag="xh")
            xl = xTpool.tile([P, DC, P], BF16, tag="xl")
            for dc in range(DC):
                pt1 = psum_t.tile([P, P], BF16, tag="pt1")
                nc.tensor.transpose(pt1, xh_nat[:, dc * P : (dc + 1) * P], ident)
                nc.scalar.copy(xh[:, dc, :], pt1)
                pt2 = psum_t.tile([P, P], BF16, tag="pt2")
                nc.tensor.transpose(pt2, xl_nat[:, dc * P : (dc + 1) * P], ident)
                nc.scalar.copy(xl[:, dc, :], pt2)

            sc = scpool.tile([P, K], F32)
            for nb in range(NB):
                terms = [(xh, eh_b[nb]), (xl, eh_b[nb]), (xh, el_b[nb])]
                bank = psum_s.tile([P, 512], F32, tag="bank")
                # bias first: its extra cost merges with the start=True penalty
                nc.tensor.matmul(
                    bank,
                    ones3,
                    bias3[:, nb * 512 : (nb + 1) * 512],
                    start=True,
                    stop=False,
                )
                n_mm = len(terms) * DC
                i = 0
                for lhsT, rhs in terms:
                    for dc in range(DC):
                        i += 1
                        nc.tensor.matmul(
                            bank,
                            lhsT[:, dc, :],
                            rhs[:, dc, :],
                            start=False,
                            stop=(i == n_mm),
                        )
                if nb < 3:
                    nc.scalar.copy(sc[:, nb * 512 : (nb + 1) * 512], bank)
                else:
                    nc.vector.tensor_copy(sc[:, nb * 512 : (nb + 1) * 512], bank)

            mx = smallpool.tile([P, 8], F32)
            nc.vector.max(out=mx, in_=sc)
            idx = smallpool.tile([P, 8], U32)
            nc.vector.max_index(out=idx, in_max=mx, in_values=sc)

            nc.sync.dma_start(
                codes[t * P : (t + 1) * P, None], idx[:, 0:1].bitcast(I32)
            )

            qt = qpool.tile([P, D], F32)
            nc.gpsimd.indirect_dma_start(
                out=qt,
                out_offset=None,
                in_=emb_dram[:, :],
                in_offset=bass.IndirectOffsetOnAxis(ap=idx[:, 0:1], axis=0),
            )
            nc.sync.dma_start(q[t * P : (t + 1) * P, :], qt)

        psum_s_pool.__exit__(None, None, None)


def build_bass(nshard):
    """Build + bacc-compile the single-core SPMD program for an nshard-point
    shard. Returns the Bass object."""
    assert nshard % P == 0
    nc = bacc.Bacc(
        "TRN2",
        target_bir_lowering=False,
        debug=False,
        enable_asserts=False,
        num_devices=N_CORES,
    )
    x = nc.dram_tensor("x", [nshard, D], F32, kind="ExternalInput")
    es = nc.dram_tensor("embedding_sum", [K, D], F32, kind="ExternalInput")
    cu = nc.dram_tensor("cluster_usage", [K], F32, kind="ExternalInput")
    q = nc.dram_tensor("quantized", [nshard, D], F32, kind="ExternalOutput")
    codes = nc.dram_tensor("codes", [nshard], I32, kind="ExternalOutput")
    emb_dram = nc.dram_tensor("emb_scratch", [K, D], F32)
    b3_dram = nc.dram_tensor("b3_scratch", [3, K], BF16)

    with tile.TileContext(nc) as tc:
        _body(
            tc,
            x.ap(),
            es.ap(),
            cu.ap(),
            q.ap(),
            codes.ap(),
            emb_dram.ap(),
            b3_dram.ap(),
            nshard // P,
        )
    nc.compile()
    return nc


_CACHE = {}
_DEVICE_POKED = False


def _poke_device():
    """A bass NEFF execution can leave the axon terminal worker in an
    unrecoverable state for the NEXT session; the first device touch from a
    fresh process fails once and restarts the worker. Absorb that in a
    sacrificial subprocess before doing real work in this process."""
    global _DEVICE_POKED
    if _DEVICE_POKED:
        return
    for _ in range(3):
        try:
            r = subprocess.run(
                [
                    sys.executable,
                    "-c",
                    "import jax, jax.numpy as jnp; "
                    "(jnp.ones((4,4)) @ jnp.ones((4,4))).block_until_ready()",
                ],
                capture_output=True,
                timeout=180,
            )
            if r.returncode == 0:
                break
        except subprocess.TimeoutExpired:
            pass
    _DEVICE_POKED = True


def _get_nc(nshard):
    if nshard not in _CACHE:
        _CACHE[nshard] = build_bass(nshard)
    return _CACHE[nshard]


def run_sharded(x, embedding_sum, cluster_usage, trace=False, nc=None):
    """Run on 8 NeuronCores; returns (quantized, codes, BassKernelResults)."""
    x = np.ascontiguousarray(np.asarray(x, dtype=np.float32))
    es = np.ascontiguousarray(np.asarray(embedding_sum, dtype=np.float32))
    cu = np.ascontiguousarray(np.asarray(cluster_usage, dtype=np.float32))
    flat = x.reshape(-1, x.shape[-1])
    n = flat.shape[0]
    assert n % N_CORES == 0
    nshard = n // N_CORES
    _poke_device()
    if nc is None:
        nc = _get_nc(nshard)

    in_maps = [
        {
            "x": np.ascontiguousarray(flat[c * nshard : (c + 1) * nshard]),
            "embedding_sum": es,
            "cluster_usage": cu,
        }
        for c in range(N_CORES)
    ]
    res = run_bass_kernel_spmd(nc, in_maps, core_ids=list(range(N_CORES)), trace=trace)
    quant = np.concatenate([r["quantized"] for r in res.results], axis=0)
    cds = np.concatenate([r["codes"] for r in res.results], axis=0)
    quant = quant.reshape(x.shape)
    cds = cds.reshape(x.shape[:-1])
    return quant, cds, res


def kernel(x, embedding_sum, cluster_usage):
    quant, cds, _ = run_sharded(x, embedding_sum, cluster_usage)
    return quant, cds
